# revision 56
# baseline (speedup 1.0000x reference)
"""GNN message-passing kernel for Trainium2 (Bass/Tile), 8-core SPMD.

Sharding: edges sharded by receiver range (edge/data parallel, no collectives).
Core c owns receivers in [c*NPC, (c+1)*NPC). Host stages a per-edge bf16
stream esT = [efT | nf[senders]T] interleaved per 128-edge tile, so the
sender gather happens at staging time and the device only streams it.

Per core:
  phase B: p2bT = (nf_loc @ W2 + b)^T kept in SBUF (fp32)
  phase C: per 128-node window w, per 128-edge tile t:
    ps  = P2b[r]^T^T                      (windowed fp32 ap_gather + fp32
                                           is_transpose into PSUM)
    ps += nfs_t @ W1 + ef_t @ W3          (two bf16 matmuls)
    msg = relu(ps) -> bf16                (ACT, batched over 4 tiles)
    S   = (iota == rank_col) -> bf16      (DVE tensor_scalar is_equal)
    agg += S^T @ msg                      (bf16 matmul, PSUM accum per window)
  out = LayerNorm(agg + nf_shard)         (fused stats on DVE/ACT)
"""

import numpy as np

import concourse.bacc as bacc
import concourse.tile as tile
import concourse.mybir as mybir
import concourse.bass as bass

F32 = mybir.dt.float32
BF16 = mybir.dt.bfloat16
I16 = mybir.dt.int16

AF = mybir.ActivationFunctionType
ALU = mybir.AluOpType


def _to_bf16(x):
    import ml_dtypes
    return x.astype(ml_dtypes.bfloat16)


# ----------------------------------------------------------------------------
# Host-side preparation
# ----------------------------------------------------------------------------

def host_prep(node_features, senders, receivers, edge_features, W, b, ln_w, ln_b,
              n_cores=8):
    N, H = node_features.shape
    E = senders.shape[0]
    assert H == 128

    node_features = np.asarray(node_features, np.float32)
    senders = np.asarray(senders, np.int32)
    receivers = np.asarray(receivers, np.int32)
    edge_features = np.asarray(edge_features, np.float32)

    # ---- balanced window sharding ------------------------------------------
    # global 128-node windows, assigned to cores by greedy bin-packing on
    # tile counts; each core's windows sorted desc into slots so all cores
    # share one slot-shape program
    NW_G = (N + 127) // 128
    win_of_edge = receivers // 128
    cnt_g = np.bincount(win_of_edge, minlength=NW_G)
    tiles_g = (cnt_g + 127) // 128
    WPC = (NW_G + n_cores - 1) // n_cores
    NPC_PAD = WPC * 128

    order_g = np.argsort(-(tiles_g * 1000000 + cnt_g), kind="stable")
    core_wins = [[] for _ in range(n_cores)]
    core_load = np.zeros(n_cores, np.int64)
    for wg in order_g:
        cands = [c for c in range(n_cores) if len(core_wins[c]) < WPC]
        c = min(cands, key=lambda c: (core_load[c], len(core_wins[c])))
        core_wins[c].append(int(wg))
        core_load[c] += int(tiles_g[wg])
    # slots: already appended in descending tile order per core
    T_w = np.zeros(WPC, np.int64)
    for j in range(WPC):
        T_w[j] = max((tiles_g[core_wins[c][j]] if j < len(core_wins[c]) else 0)
                     for c in range(n_cores))
    NT = int(T_w.sum())
    E_PAD = NT * 128
    tile_base = np.concatenate(([0], np.cumsum(T_w)[:-1]))

    ln_trivial = bool(np.allclose(ln_w, 1.0) and np.allclose(ln_b, 0.0))

    structure = dict(N=N, H=H, E=E, WPC=WPC, NPC_PAD=NPC_PAD,
                     NT=NT, E_PAD=E_PAD, T_w=T_w, tile_base=tile_base,
                     ln_trivial=ln_trivial, core_wins=core_wins)

    def wrap_idx(arr):
        """int16 stream -> [128, L/16] wrapped layout (replicated per 16 rows)."""
        L = arr.shape[0]
        assert L % 16 == 0
        w16 = arr.reshape(-1, 16).T.astype(np.int16)   # [16, L/16]
        return np.ascontiguousarray(np.tile(w16, (8, 1)))

    # ---- shared (core-independent) inputs -----------------------------------
    W1 = np.ascontiguousarray(W[0:128], np.float32)
    W2 = np.ascontiguousarray(W[128:256], np.float32)
    W3 = np.ascontiguousarray(W[256:384], np.float32)
    iota_row = np.broadcast_to(np.arange(128, dtype=np.float32), (128, 128))
    shared = {
        "W1b": _to_bf16(W1),
        "W2b": _to_bf16(W2),
        "W3b": _to_bf16(W3),
        "b_col": np.asarray(b, np.float32).reshape(128, 1).copy(),
        "iota16": _to_bf16(iota_row.copy()),
        "ident": np.eye(128, dtype=np.float32),
        "lnw_bc": np.broadcast_to(np.asarray(ln_w, np.float32), (128, 128)).copy(),
        "lnb_bc": np.broadcast_to(np.asarray(ln_b, np.float32), (128, 128)).copy(),
    }

    # ---- pass 2: per-core streams (slot order) ------------------------------
    # edges sorted globally by receiver window once; then assembled per core
    order_e = np.argsort(win_of_edge, kind="stable")
    win_sorted = win_of_edge[order_e]
    wstart = np.searchsorted(win_sorted, np.arange(NW_G))
    wend = np.searchsorted(win_sorted, np.arange(NW_G) + 1)

    in_maps = []
    for c in range(n_cores):
        wins = core_wins[c]
        sel_parts, rank_parts, slot_parts = [], [], []
        for j, wg in enumerate(wins):
            e_ids = order_e[wstart[wg]:wend[wg]]
            sel_parts.append(e_ids)
            rank_parts.append(receivers[e_ids] - wg * 128)
            slot_parts.append(np.full(e_ids.shape[0], j, np.int64))
        sel = np.concatenate(sel_parts)
        r_rank = np.concatenate(rank_parts)
        slot = np.concatenate(slot_parts)
        Ec = sel.shape[0]
        # position of each edge in the padded stream
        starts = np.concatenate(([0], np.nonzero(np.diff(slot))[0] + 1))
        grp_start = np.repeat(starts, np.diff(np.concatenate((starts, [Ec]))))
        jj = np.arange(Ec) - grp_start
        pos = tile_base[slot] * 128 + jj

        ef_pad = np.zeros((E_PAD, 128), np.float32)
        ef_pad[pos] = edge_features[sel]
        nfs_pad = np.zeros((E_PAD, 128), np.float32)
        nfs_pad[pos] = node_features[senders[sel]]

        # esT [128, 2*E_PAD] bf16: per tile t, cols [2t*128,(2t+1)*128) = efT,
        # cols [(2t+1)*128,(2t+2)*128) = nf[s]T
        esT = np.empty((128, NT, 2, 128), np.float32)
        esT[:, :, 0, :] = ef_pad.reshape(NT, 128, 128).transpose(2, 0, 1)
        esT[:, :, 1, :] = nfs_pad.reshape(NT, 128, 128).transpose(2, 0, 1)
        esT = _to_bf16(np.ascontiguousarray(esT.reshape(128, 2 * E_PAD)))

        rank = np.full(E_PAD, -1.0, np.float32)
        rank[pos] = r_rank.astype(np.float32)
        rankT = np.ascontiguousarray(rank.reshape(NT, 128).T)   # [128, NT]

        idx_rank = np.zeros(E_PAD, np.int64)
        idx_rank[pos] = r_rank

        # node rows per slot (window 390 may extend past N: zero-pad)
        nf_shard = np.zeros((NPC_PAD, 128), np.float32)
        for j, wg in enumerate(wins):
            lo, hi = wg * 128, min((wg + 1) * 128, N)
            nf_shard[j * 128:j * 128 + hi - lo] = node_features[lo:hi]
        nfT_loc = np.ascontiguousarray(nf_shard.T)

        m = dict(shared)
        m.update({
            "esT": esT,
            "rankT": rankT,
            "idx_rank": wrap_idx(idx_rank),
            "nf_shard": nf_shard,
            "nfT_loc": _to_bf16(nfT_loc),
        })
        in_maps.append(m)

    return structure, in_maps


# ----------------------------------------------------------------------------
# Bass kernel builder
# ----------------------------------------------------------------------------

def build_kernel(st, eps=1e-5, max_windows=None):
    NPC_PAD, WPC = st["NPC_PAD"], st["WPC"]
    NT, E_PAD = st["NT"], st["E_PAD"]
    T_w, tile_base = st["T_w"], st["tile_base"]
    ln_trivial = st["ln_trivial"]
    T_MAX = int(T_w.max())
    # es load batches: one window per batch (fast pipeline priming)
    batches = [(w, 1) for w in range(WPC)]
    batch_of = {}
    for bi, (bw, bn) in enumerate(batches):
        for j in range(bn):
            batch_of[bw + j] = (bi, bw)
    T2 = [int(sum(T_w[bw:bw + bn])) for (bw, bn) in batches]
    T2_MAX = max(T2)
    # idx table split: first 8 windows separate so window 0's gather isn't
    # gated by the full index upload
    W_SPLIT = min(8, WPC)
    IDX_SPLIT = int(tile_base[W_SPLIT] * 8) if W_SPLIT < WPC else E_PAD // 16

    nc = bacc.Bacc("TRN2", target_bir_lowering=False, debug=False)

    # inputs
    esT = nc.dram_tensor("esT", [128, 2 * E_PAD], BF16, kind="ExternalInput")
    rankT = nc.dram_tensor("rankT", [128, NT], F32, kind="ExternalInput")
    idx_rank = nc.dram_tensor("idx_rank", [128, E_PAD // 16], I16,
                              kind="ExternalInput")
    nfT_loc = nc.dram_tensor("nfT_loc", [128, NPC_PAD], BF16, kind="ExternalInput")
    nf_shard = nc.dram_tensor("nf_shard", [NPC_PAD, 128], F32, kind="ExternalInput")
    W1b = nc.dram_tensor("W1b", [128, 128], BF16, kind="ExternalInput")
    W2b = nc.dram_tensor("W2b", [128, 128], BF16, kind="ExternalInput")
    W3b = nc.dram_tensor("W3b", [128, 128], BF16, kind="ExternalInput")
    b_col = nc.dram_tensor("b_col", [128, 1], F32, kind="ExternalInput")
    iota16 = nc.dram_tensor("iota16", [128, 128], BF16, kind="ExternalInput")
    ident = nc.dram_tensor("ident", [128, 128], F32, kind="ExternalInput")
    lnw_bc = nc.dram_tensor("lnw_bc", [128, 128], F32, kind="ExternalInput")
    lnb_bc = nc.dram_tensor("lnb_bc", [128, 128], F32, kind="ExternalInput")

    out_shard = nc.dram_tensor("out_shard", [NPC_PAD, 128], F32,
                               kind="ExternalOutput")

    with tile.TileContext(nc) as tc:
        with (
            tc.tile_pool(name="consts", bufs=1) as consts,
        ):
            # constants in SBUF
            W1_sb = consts.tile([128, 128], BF16)
            W2_sb = consts.tile([128, 128], BF16)
            W3_sb = consts.tile([128, 128], BF16)
            iota_sb = consts.tile([128, 128], BF16)
            ident_sb = consts.tile([128, 128], F32)
            bcol_sb = consts.tile([128, 1], F32)
            lnw_sb = consts.tile([128, 128], F32)
            lnb_sb = consts.tile([128, 128], F32)
            eps_sb = consts.tile([128, 1], F32)
            rankT_a = consts.tile([128, IDX_SPLIT // 8], F32)
            rankT_b = consts.tile([128, NT - IDX_SPLIT // 8], F32)
            idxr_a = consts.tile([128, IDX_SPLIT], I16)
            idxr_b = consts.tile([128, E_PAD // 16 - IDX_SPLIT], I16)
            # nf_loc in 2048-col pieces so phase B can start on piece 0
            NFL_P = 2048
            nfl_ps = [consts.tile([128, min(NFL_P, NPC_PAD - i)], BF16,
                                  name=f"nfl{i // NFL_P}")
                      for i in range(0, NPC_PAD, NFL_P)]
            # one tile per 512-node chunk so early windows' gathers only
            # depend on their own chunk of phase B
            NCHUNK = (NPC_PAD + 511) // 512
            p2bT_chunks = [consts.tile([128, 512], F32, name=f"p2bT{i}")
                           for i in range(NCHUNK)]

            # issue order tuned for the critical path: W2 first, then the
            # first es batch + its indices, then nf_loc pieces; small consts
            # via the ACT queue
            nc.sync.dma_start(out=W2_sb[:], in_=W2b[:])
            nc.sync.dma_start(out=bcol_sb[:], in_=b_col[:])
            nc.sync.dma_start(out=idxr_a[:], in_=idx_rank[:, :IDX_SPLIT])
            nc.sync.dma_start(out=nfl_ps[0][:],
                              in_=nfT_loc[:, :min(NFL_P, NPC_PAD)])
            nc.scalar.dma_start(out=W1_sb[:], in_=W1b[:])
            nc.scalar.dma_start(out=W3_sb[:], in_=W3b[:])
            nc.scalar.dma_start(out=iota_sb[:], in_=iota16[:])
            nc.scalar.dma_start(out=ident_sb[:], in_=ident[:])
            nc.scalar.dma_start(out=rankT_a[:], in_=rankT[:, :IDX_SPLIT // 8])
            nc.scalar.dma_start(out=rankT_b[:], in_=rankT[:, IDX_SPLIT // 8:])
            if not ln_trivial:
                nc.scalar.dma_start(out=lnw_sb[:], in_=lnw_bc[:])
                nc.scalar.dma_start(out=lnb_sb[:], in_=lnb_bc[:])
            nc.vector.memset(eps_sb[:], eps)

            # ---- phase C (with phase B interleaved lazily) ------------------
            with (
                tc.tile_pool(name="es", bufs=3) as espool,
                tc.tile_pool(name="p2xp", bufs=2) as p2xp,
                tc.tile_pool(name="msgs", bufs=4) as msgs,
                tc.tile_pool(name="pbps", bufs=1, space="PSUM") as pbps,
                tc.tile_pool(name="ppsum", bufs=4, space="PSUM") as ppsum,
                tc.tile_pool(name="aggp", bufs=3, space="PSUM") as aggp,
                tc.tile_pool(name="wt", bufs=4) as wt,
                tc.tile_pool(name="outp", bufs=3) as outp,
            ):
                # phase B: p2bT = (nf_loc @ W2 + b)^T, one 512-col chunk at a
                # time, emitted lazily from the window loop
                next_chunk = [0]

                def emit_pb_chunks(upto):
                    while next_chunk[0] <= min(upto, NCHUNK - 1):
                        j0 = next_chunk[0] * 512
                        k = min(512, NPC_PAD - j0)
                        pc = nfl_ps[j0 // NFL_P]
                        poff = j0 % NFL_P
                        psA = pbps.tile([128, 512], F32, tag="psA")
                        nc.tensor.matmul(out=psA[:, :k], lhsT=W2_sb[:],
                                         rhs=pc[:, poff:poff + k],
                                         start=True, stop=True)
                        nc.vector.tensor_scalar(
                            out=p2bT_chunks[next_chunk[0]][:, :k],
                            in0=psA[:, :k], scalar1=bcol_sb[:],
                            scalar2=None, op0=ALU.add)
                        next_chunk[0] += 1
                n_win = WPC if max_windows is None else min(max_windows, WPC)

                def emit_ln(w, agg, ot_batch):
                    """Residual + LayerNorm + batched store for window w."""
                    tw = int(T_w[w])
                    nf_w = wt.tile([128, 128], F32, tag="nfw")
                    nc.scalar.dma_start(out=nf_w[:],
                                        in_=nf_shard[w * 128:(w + 1) * 128, :])
                    x = wt.tile([128, 128], F32, tag="x")
                    s1 = wt.tile([128, 1], F32, tag="s1")
                    if tw > 0:
                        nc.vector.scalar_tensor_tensor(
                            out=x[:], in0=agg[:], scalar=0.0, in1=nf_w[:],
                            op0=ALU.add, op1=ALU.add, accum_out=s1[:])
                    else:
                        nc.vector.scalar_tensor_tensor(
                            out=x[:], in0=nf_w[:], scalar=0.0, in1=nf_w[:],
                            op0=ALU.mult, op1=ALU.add, accum_out=s1[:])
                    xsq = wt.tile([128, 128], F32, tag="xsq")
                    s2 = wt.tile([128, 1], F32, tag="s2")
                    nc.vector.scalar_tensor_tensor(
                        out=xsq[:], in0=x[:], scalar=1.0, in1=x[:],
                        op0=ALU.mult, op1=ALU.mult, accum_out=s2[:])
                    mu = wt.tile([128, 1], F32, tag="mu")
                    nc.vector.tensor_scalar(
                        out=mu[:], in0=s1[:], scalar1=1.0 / 128.0,
                        scalar2=None, op0=ALU.mult)
                    mu2 = wt.tile([128, 1], F32, tag="mu2")
                    nc.vector.tensor_mul(out=mu2[:], in0=mu[:], in1=mu[:])
                    var = wt.tile([128, 1], F32, tag="var")
                    nc.vector.tensor_scalar(
                        out=var[:], in0=s2[:], scalar1=1.0 / 128.0,
                        scalar2=mu2[:], op0=ALU.mult, op1=ALU.subtract)
                    sd = wt.tile([128, 1], F32, tag="sd")
                    nc.scalar.activation(out=sd[:], in_=var[:], func=AF.Sqrt,
                                         bias=eps_sb[:], scale=1.0)
                    rs = wt.tile([128, 1], F32, tag="rs")
                    nc.vector.reciprocal(out=rs[:], in_=sd[:])
                    if ln_trivial:
                        nc.vector.tensor_scalar(
                            out=ot_batch[:, w % 4, :], in0=x[:],
                            scalar1=mu[:], scalar2=rs[:],
                            op0=ALU.subtract, op1=ALU.mult)
                    else:
                        xn = wt.tile([128, 128], F32, tag="xn")
                        nc.vector.tensor_scalar(
                            out=xn[:], in0=x[:],
                            scalar1=mu[:], scalar2=rs[:],
                            op0=ALU.subtract, op1=ALU.mult)
                        xw = wt.tile([128, 128], F32, tag="xw")
                        nc.vector.tensor_mul(out=xw[:], in0=xn[:], in1=lnw_sb[:])
                        nc.vector.tensor_add(out=ot_batch[:, w % 4, :],
                                             in0=xw[:], in1=lnb_sb[:])
                    if w % 4 == 3 or w == n_win - 1:
                        w0 = (w // 4) * 4
                        kw = w - w0 + 1
                        dst = out_shard[w0 * 128:(w0 + kw) * 128, :].rearrange(
                            "(t p) h -> p t h", p=128)
                        nc.scalar.dma_start(out=dst, in_=ot_batch[:, :kw, :])

                # software pipeline: agg matmuls of group i emitted after the
                # accumulation matmuls of group i+1 so the PE queue never
                # stalls at an agg waiting on ACT's relu
                pending = None       # (w, agg, S, msg, c0, k, t0, tw, ot)

                def flush_pending():
                    nonlocal pending
                    if pending is None:
                        return
                    (pw, agg, S, msg, c0, k, t0, ptw, pot) = pending
                    for t in range(k):
                        nc.tensor.matmul(
                            out=agg[:], lhsT=S[:, c0 + t, :], rhs=msg[:, t, :],
                            start=(t0 + t == 0), stop=(t0 + t == ptw - 1),
                            skip_group_check=True)
                    pending = None
                    if c0 + k == ptw:
                        emit_ln(pw, agg, pot)

                es_sb = None
                ot_batch = None
                es0_subs = None
                for w in range(n_win):
                    tw = int(T_w[w])
                    tb = int(tile_base[w])
                    if w == 0:
                        # window 0 es in two halves so PE starts sooner;
                        # remaining nf_loc pieces + bulk indices after
                        half = (tw + 1) // 2
                        es0_subs = []
                        for si in (0, half):
                            sn = min(half, tw - si)
                            sub = espool.tile([128, 2 * half * 128], BF16,
                                              name=f"es0s{si}")
                            nc.sync.dma_start(
                                out=sub[:, :2 * sn * 128],
                                in_=esT[:, 2 * si * 128:2 * (si + sn) * 128])
                            es0_subs.append((sub, si, sn))
                    elif w in batch_of and batch_of[w][1] == w:
                        # load es for this batch's windows
                        bi, bw = batch_of[w]
                        bt = T2[bi]
                        es_sb = espool.tile([128, 2 * T2_MAX * 128], BF16,
                                            tag="es")
                        nc.sync.dma_start(
                            out=es_sb[:, :2 * bt * 128],
                            in_=esT[:, 2 * tb * 128:2 * (tb + bt) * 128])
                        es_off = 0
                    else:
                        es_off = 2 * int(T_w[w - 1]) * 128
                    if 1 <= w <= 3:
                        # deferred bulk loads, not needed until window 8/16+
                        pi = w
                        pk = min(NFL_P, NPC_PAD - pi * NFL_P)
                        if pk > 0:
                            nc.sync.dma_start(
                                out=nfl_ps[pi][:],
                                in_=nfT_loc[:, pi * NFL_P:pi * NFL_P + pk])
                        if w == 3:
                            nc.sync.dma_start(out=idxr_b[:],
                                              in_=idx_rank[:, IDX_SPLIT:])

                    # output batch tile (4 windows per store)
                    if w % 4 == 0:
                        ot_batch = outp.tile([128, 4, 128], F32, tag="ot")

                    if tw == 0:
                        flush_pending()
                        emit_ln(w, None, ot_batch)
                        continue

                    # phase B chunks needed by this window's gather (+1 ahead)
                    emit_pb_chunks(w // 4 + 1)

                    chunk = p2bT_chunks[(w * 128) // 512]
                    coff = (w * 128) % 512
                    if w < W_SPLIT:
                        idxs_ap = idxr_a[:, tb * 8:(tb + tw) * 8]
                    else:
                        idxs_ap = idxr_b[:, tb * 8 - IDX_SPLIT:
                                         (tb + tw) * 8 - IDX_SPLIT]
                    p2x = p2xp.tile([128, T_MAX * 128], F32, tag="p2x")
                    nc.gpsimd.ap_gather(
                        out_ap=p2x[:, :tw * 128].rearrange(
                            "p (n d) -> p n d", d=1),
                        in_ap=chunk[:, coff:coff + 128].rearrange(
                            "p (n d) -> p n d", d=1),
                        idxs_ap=idxs_ap,
                        channels=128, num_elems=128, d=1,
                        num_idxs=tw * 128)

                    # S depends only on consts: generate whole window early
                    S = msgs.tile([128, T_MAX, 128], BF16, tag="S")
                    for t in range(tw):
                        nc.vector.tensor_scalar(
                            out=S[:, t, :], in0=iota_sb[:],
                            scalar1=(rankT_a[:, tb + t:tb + t + 1]
                                     if w < W_SPLIT else
                                     rankT_b[:, tb + t - IDX_SPLIT // 8:
                                             tb + t + 1 - IDX_SPLIT // 8]),
                            scalar2=None, op0=ALU.is_equal)

                    def es_ap(g, block):
                        """AP for tile g's ef (block=0) / nfs (block=1)."""
                        if w == 0:
                            for sub, si, sn in es0_subs:
                                if si <= g < si + sn:
                                    o = 2 * (g - si) + block
                                    return sub[:, o * 128:(o + 1) * 128]
                            raise AssertionError
                        o = es_off + (2 * g + block) * 128
                        return es_sb[:, o:o + 128]

                    agg = aggp.tile([128, 128], F32, tag="agg")
                    t_done = 0
                    for c0 in range(0, tw, 4):
                        k = min(4, tw - c0)
                        ps = ppsum.tile([128, 4, 128], F32, tag="ps")
                        for t in range(k):
                            g = c0 + t
                            nc.tensor.matmul(
                                out=ps[:, t, :],
                                lhsT=es_ap(g, 1),
                                rhs=W1_sb[:],
                                start=True, stop=False,
                                skip_group_check=True)
                            nc.tensor.matmul(
                                out=ps[:, t, :],
                                lhsT=es_ap(g, 0),
                                rhs=W3_sb[:],
                                start=False, stop=False,
                                skip_group_check=True)
                            nc.tensor.matmul(
                                out=ps[:, t, :],
                                lhsT=p2x[:, g * 128:(g + 1) * 128],
                                rhs=ident_sb[:],
                                is_transpose=True,
                                start=False, stop=True,
                                skip_group_check=True)
                        flush_pending()
                        msg = msgs.tile([128, 4, 128], BF16, tag="msg")
                        nc.scalar.activation(
                            out=msg[:, :k, :], in_=ps[:, :k, :],
                            func=AF.Relu, scale=1.0)
                        pending = (w, agg, S, msg, c0, k, t_done, tw, ot_batch)
                        t_done += k
                flush_pending()

    nc.compile()
    return nc


# ----------------------------------------------------------------------------
# Full entry: host prep + device run + assembly
# ----------------------------------------------------------------------------

def run(node_features, senders, receivers, edge_features, W, b, ln_w, ln_b,
        n_cores=8, return_nc=False):
    from concourse.bass_utils import run_bass_kernel_spmd
    st, in_maps = host_prep(node_features, senders, receivers, edge_features,
                            W, b, ln_w, ln_b, n_cores)
    nc = build_kernel(st)
    res = run_bass_kernel_spmd(nc, in_maps, core_ids=list(range(n_cores)))
    N = st["N"]
    out = np.empty((N, 128), np.float32)
    for c in range(n_cores):
        shard = res.results[c]["out_shard"]
        for j, wg in enumerate(st["core_wins"][c]):
            lo, hi = wg * 128, min((wg + 1) * 128, N)
            out[lo:hi] = shard[j * 128:j * 128 + hi - lo]
    if return_nc:
        return out, nc, st, in_maps
    return out


# ----------------------------------------------------------------------------
# Harness entry point
# ----------------------------------------------------------------------------

def kernel(**inputs):
    """Full-input entry: shards across 8 NeuronCores internally."""
    out = run(
        node_features=np.asarray(inputs["node_features"], np.float32),
        senders=np.asarray(inputs["senders"], np.int32),
        receivers=np.asarray(inputs["receivers"], np.int32),
        edge_features=np.asarray(inputs["edge_features"], np.float32),
        W=np.asarray(inputs["W"], np.float32),
        b=np.asarray(inputs["b"], np.float32),
        ln_w=np.asarray(inputs["ln_w"], np.float32),
        ln_b=np.asarray(inputs["ln_b"], np.float32),
        n_cores=8,
    )
    return out.astype(np.float32)


# revision 58
# speedup vs baseline: 1.0023x; 1.0023x over previous
"""GNN message-passing kernel for Trainium2 (Bass/Tile), 8-core SPMD.

Sharding: edges sharded by receiver range (edge/data parallel, no collectives).
Core c owns receivers in [c*NPC, (c+1)*NPC). Host stages a per-edge bf16
stream esT = [efT | nf[senders]T] interleaved per 128-edge tile, so the
sender gather happens at staging time and the device only streams it.

Per core:
  phase B: p2bT = (nf_loc @ W2 + b)^T kept in SBUF (fp32)
  phase C: per 128-node window w, per 128-edge tile t:
    ps  = P2b[r]^T^T                      (windowed fp32 ap_gather + fp32
                                           is_transpose into PSUM)
    ps += nfs_t @ W1 + ef_t @ W3          (two bf16 matmuls)
    msg = relu(ps) -> bf16                (ACT, batched over 4 tiles)
    S   = (iota == rank_col) -> bf16      (DVE tensor_scalar is_equal)
    agg += S^T @ msg                      (bf16 matmul, PSUM accum per window)
  out = LayerNorm(agg + nf_shard)         (fused stats on DVE/ACT)
"""

import numpy as np

import concourse.bacc as bacc
import concourse.tile as tile
import concourse.mybir as mybir
import concourse.bass as bass

F32 = mybir.dt.float32
BF16 = mybir.dt.bfloat16
I16 = mybir.dt.int16

AF = mybir.ActivationFunctionType
ALU = mybir.AluOpType


def _to_bf16(x):
    import ml_dtypes
    return x.astype(ml_dtypes.bfloat16)


# ----------------------------------------------------------------------------
# Host-side preparation
# ----------------------------------------------------------------------------

def host_prep(node_features, senders, receivers, edge_features, W, b, ln_w, ln_b,
              n_cores=8):
    N, H = node_features.shape
    E = senders.shape[0]
    assert H == 128

    node_features = np.asarray(node_features, np.float32)
    senders = np.asarray(senders, np.int32)
    receivers = np.asarray(receivers, np.int32)
    edge_features = np.asarray(edge_features, np.float32)

    # ---- balanced window sharding ------------------------------------------
    # global 128-node windows, assigned to cores by greedy bin-packing on
    # tile counts; each core's windows sorted desc into slots so all cores
    # share one slot-shape program
    NW_G = (N + 127) // 128
    win_of_edge = receivers // 128
    cnt_g = np.bincount(win_of_edge, minlength=NW_G)
    tiles_g = (cnt_g + 127) // 128
    WPC = (NW_G + n_cores - 1) // n_cores
    NPC_PAD = WPC * 128

    order_g = np.argsort(-(tiles_g * 1000000 + cnt_g), kind="stable")
    core_wins = [[] for _ in range(n_cores)]
    core_load = np.zeros(n_cores, np.int64)
    for wg in order_g:
        cands = [c for c in range(n_cores) if len(core_wins[c]) < WPC]
        c = min(cands, key=lambda c: (core_load[c], len(core_wins[c])))
        core_wins[c].append(int(wg))
        core_load[c] += int(tiles_g[wg])
    # slots: already appended in descending tile order per core
    T_w = np.zeros(WPC, np.int64)
    for j in range(WPC):
        T_w[j] = max((tiles_g[core_wins[c][j]] if j < len(core_wins[c]) else 0)
                     for c in range(n_cores))
    NT = int(T_w.sum())
    E_PAD = NT * 128
    tile_base = np.concatenate(([0], np.cumsum(T_w)[:-1]))

    ln_trivial = bool(np.allclose(ln_w, 1.0) and np.allclose(ln_b, 0.0))

    structure = dict(N=N, H=H, E=E, WPC=WPC, NPC_PAD=NPC_PAD,
                     NT=NT, E_PAD=E_PAD, T_w=T_w, tile_base=tile_base,
                     ln_trivial=ln_trivial, core_wins=core_wins)

    def wrap_idx(arr):
        """int16 stream -> [128, L/16] wrapped layout (replicated per 16 rows)."""
        L = arr.shape[0]
        assert L % 16 == 0
        w16 = arr.reshape(-1, 16).T.astype(np.int16)   # [16, L/16]
        return np.ascontiguousarray(np.tile(w16, (8, 1)))

    # ---- shared (core-independent) inputs -----------------------------------
    W1 = np.ascontiguousarray(W[0:128], np.float32)
    W2 = np.ascontiguousarray(W[128:256], np.float32)
    W3 = np.ascontiguousarray(W[256:384], np.float32)
    iota_row = np.broadcast_to(np.arange(128, dtype=np.float32), (128, 128))
    shared = {
        "W1b": _to_bf16(W1),
        "W2b": _to_bf16(W2),
        "W3b": _to_bf16(W3),
        "b_col": np.asarray(b, np.float32).reshape(128, 1).copy(),
        "iota16": _to_bf16(iota_row.copy()),
        "ident": np.eye(128, dtype=np.float32),
        "lnw_bc": np.broadcast_to(np.asarray(ln_w, np.float32), (128, 128)).copy(),
        "lnb_bc": np.broadcast_to(np.asarray(ln_b, np.float32), (128, 128)).copy(),
    }

    # ---- pass 2: per-core streams (slot order) ------------------------------
    # edges sorted globally by receiver window once; then assembled per core
    order_e = np.argsort(win_of_edge, kind="stable")
    win_sorted = win_of_edge[order_e]
    wstart = np.searchsorted(win_sorted, np.arange(NW_G))
    wend = np.searchsorted(win_sorted, np.arange(NW_G) + 1)

    in_maps = []
    for c in range(n_cores):
        wins = core_wins[c]
        sel_parts, rank_parts, slot_parts = [], [], []
        for j, wg in enumerate(wins):
            e_ids = order_e[wstart[wg]:wend[wg]]
            sel_parts.append(e_ids)
            rank_parts.append(receivers[e_ids] - wg * 128)
            slot_parts.append(np.full(e_ids.shape[0], j, np.int64))
        sel = np.concatenate(sel_parts)
        r_rank = np.concatenate(rank_parts)
        slot = np.concatenate(slot_parts)
        Ec = sel.shape[0]
        # position of each edge in the padded stream
        starts = np.concatenate(([0], np.nonzero(np.diff(slot))[0] + 1))
        grp_start = np.repeat(starts, np.diff(np.concatenate((starts, [Ec]))))
        jj = np.arange(Ec) - grp_start
        pos = tile_base[slot] * 128 + jj

        ef_pad = np.zeros((E_PAD, 128), np.float32)
        ef_pad[pos] = edge_features[sel]
        nfs_pad = np.zeros((E_PAD, 128), np.float32)
        nfs_pad[pos] = node_features[senders[sel]]

        # esT [128, 2*E_PAD] bf16: per tile t, cols [2t*128,(2t+1)*128) = efT,
        # cols [(2t+1)*128,(2t+2)*128) = nf[s]T
        esT = np.empty((128, NT, 2, 128), np.float32)
        esT[:, :, 0, :] = ef_pad.reshape(NT, 128, 128).transpose(2, 0, 1)
        esT[:, :, 1, :] = nfs_pad.reshape(NT, 128, 128).transpose(2, 0, 1)
        esT = _to_bf16(np.ascontiguousarray(esT.reshape(128, 2 * E_PAD)))

        rank = np.full(E_PAD, -1.0, np.float32)
        rank[pos] = r_rank.astype(np.float32)
        rankT = np.ascontiguousarray(rank.reshape(NT, 128).T)   # [128, NT]

        idx_rank = np.zeros(E_PAD, np.int64)
        idx_rank[pos] = r_rank

        # node rows per slot (window 390 may extend past N: zero-pad)
        nf_shard = np.zeros((NPC_PAD, 128), np.float32)
        for j, wg in enumerate(wins):
            lo, hi = wg * 128, min((wg + 1) * 128, N)
            nf_shard[j * 128:j * 128 + hi - lo] = node_features[lo:hi]
        nfT_loc = np.ascontiguousarray(nf_shard.T)

        m = dict(shared)
        m.update({
            "esT": esT,
            "rankT": rankT,
            "idx_rank": wrap_idx(idx_rank),
            "nf_shard": nf_shard,
            "nfT_loc": _to_bf16(nfT_loc),
        })
        in_maps.append(m)

    return structure, in_maps


# ----------------------------------------------------------------------------
# Bass kernel builder
# ----------------------------------------------------------------------------

def build_kernel(st, eps=1e-5, max_windows=None):
    NPC_PAD, WPC = st["NPC_PAD"], st["WPC"]
    NT, E_PAD = st["NT"], st["E_PAD"]
    T_w, tile_base = st["T_w"], st["tile_base"]
    ln_trivial = st["ln_trivial"]
    T_MAX = int(T_w.max())
    # es load batches: one window per batch (fast pipeline priming)
    batches = [(w, 1) for w in range(WPC)]
    batch_of = {}
    for bi, (bw, bn) in enumerate(batches):
        for j in range(bn):
            batch_of[bw + j] = (bi, bw)
    T2 = [int(sum(T_w[bw:bw + bn])) for (bw, bn) in batches]
    T2_MAX = max(T2)
    # idx table split: first 8 windows separate so window 0's gather isn't
    # gated by the full index upload
    W_SPLIT = min(8, WPC)
    IDX_SPLIT = int(tile_base[W_SPLIT] * 8) if W_SPLIT < WPC else E_PAD // 16

    nc = bacc.Bacc("TRN2", target_bir_lowering=False, debug=False)

    # inputs
    esT = nc.dram_tensor("esT", [128, 2 * E_PAD], BF16, kind="ExternalInput")
    rankT = nc.dram_tensor("rankT", [128, NT], F32, kind="ExternalInput")
    idx_rank = nc.dram_tensor("idx_rank", [128, E_PAD // 16], I16,
                              kind="ExternalInput")
    nfT_loc = nc.dram_tensor("nfT_loc", [128, NPC_PAD], BF16, kind="ExternalInput")
    nf_shard = nc.dram_tensor("nf_shard", [NPC_PAD, 128], F32, kind="ExternalInput")
    W1b = nc.dram_tensor("W1b", [128, 128], BF16, kind="ExternalInput")
    W2b = nc.dram_tensor("W2b", [128, 128], BF16, kind="ExternalInput")
    W3b = nc.dram_tensor("W3b", [128, 128], BF16, kind="ExternalInput")
    b_col = nc.dram_tensor("b_col", [128, 1], F32, kind="ExternalInput")
    iota16 = nc.dram_tensor("iota16", [128, 128], BF16, kind="ExternalInput")
    ident = nc.dram_tensor("ident", [128, 128], F32, kind="ExternalInput")
    lnw_bc = nc.dram_tensor("lnw_bc", [128, 128], F32, kind="ExternalInput")
    lnb_bc = nc.dram_tensor("lnb_bc", [128, 128], F32, kind="ExternalInput")

    out_shard = nc.dram_tensor("out_shard", [NPC_PAD, 128], F32,
                               kind="ExternalOutput")

    with tile.TileContext(nc) as tc:
        with (
            tc.tile_pool(name="consts", bufs=1) as consts,
        ):
            # constants in SBUF
            W1_sb = consts.tile([128, 128], BF16)
            W2_sb = consts.tile([128, 128], BF16)
            W3_sb = consts.tile([128, 128], BF16)
            iota_sb = consts.tile([128, 128], BF16)
            ident_sb = consts.tile([128, 128], F32)
            bcol_sb = consts.tile([128, 1], F32)
            lnw_sb = consts.tile([128, 128], F32)
            lnb_sb = consts.tile([128, 128], F32)
            eps_sb = consts.tile([128, 1], F32)
            rankT_a = consts.tile([128, IDX_SPLIT // 8], F32)
            rankT_b = consts.tile([128, NT - IDX_SPLIT // 8], F32)
            idxr_a = consts.tile([128, IDX_SPLIT], I16)
            idxr_b = consts.tile([128, E_PAD // 16 - IDX_SPLIT], I16)
            # nf_loc in 2048-col pieces so phase B can start on piece 0
            NFL_P = 2048
            nfl_ps = [consts.tile([128, min(NFL_P, NPC_PAD - i)], BF16,
                                  name=f"nfl{i // NFL_P}")
                      for i in range(0, NPC_PAD, NFL_P)]
            # one tile per 512-node chunk so early windows' gathers only
            # depend on their own chunk of phase B
            NCHUNK = (NPC_PAD + 511) // 512
            p2bT_chunks = [consts.tile([128, 512], F32, name=f"p2bT{i}")
                           for i in range(NCHUNK)]

            # issue order tuned for the critical path: W2 first, then the
            # first es batch + its indices, then nf_loc pieces; small consts
            # via the ACT queue
            nc.sync.dma_start(out=W2_sb[:], in_=W2b[:])
            nc.sync.dma_start(out=bcol_sb[:], in_=b_col[:])
            nc.sync.dma_start(out=idxr_a[:], in_=idx_rank[:, :IDX_SPLIT])
            nc.sync.dma_start(out=nfl_ps[0][:],
                              in_=nfT_loc[:, :min(NFL_P, NPC_PAD)])
            nc.scalar.dma_start(out=W1_sb[:], in_=W1b[:])
            nc.scalar.dma_start(out=W3_sb[:], in_=W3b[:])
            nc.scalar.dma_start(out=iota_sb[:], in_=iota16[:])
            nc.scalar.dma_start(out=ident_sb[:], in_=ident[:])
            nc.scalar.dma_start(out=rankT_a[:], in_=rankT[:, :IDX_SPLIT // 8])
            nc.scalar.dma_start(out=rankT_b[:], in_=rankT[:, IDX_SPLIT // 8:])
            if not ln_trivial:
                nc.scalar.dma_start(out=lnw_sb[:], in_=lnw_bc[:])
                nc.scalar.dma_start(out=lnb_sb[:], in_=lnb_bc[:])
            nc.vector.memset(eps_sb[:], eps)

            # ---- phase C (with phase B interleaved lazily) ------------------
            with (
                tc.tile_pool(name="es", bufs=3) as espool,
                tc.tile_pool(name="p2xp", bufs=2) as p2xp,
                tc.tile_pool(name="msgs", bufs=4) as msgs,
                tc.tile_pool(name="pbps", bufs=2, space="PSUM") as pbps,
                tc.tile_pool(name="ppsum", bufs=3, space="PSUM") as ppsum,
                tc.tile_pool(name="aggp", bufs=3, space="PSUM") as aggp,
                tc.tile_pool(name="wt", bufs=6) as wt,
                tc.tile_pool(name="outp", bufs=3) as outp,
            ):
                # phase B: p2bT = (nf_loc @ W2 + b)^T, one 512-col chunk at a
                # time, emitted lazily from the window loop
                next_chunk = [0]

                def emit_pb_chunks(upto):
                    while next_chunk[0] <= min(upto, NCHUNK - 1):
                        j0 = next_chunk[0] * 512
                        k = min(512, NPC_PAD - j0)
                        pc = nfl_ps[j0 // NFL_P]
                        poff = j0 % NFL_P
                        psA = pbps.tile([128, 512], F32, tag="psA")
                        nc.tensor.matmul(out=psA[:, :k], lhsT=W2_sb[:],
                                         rhs=pc[:, poff:poff + k],
                                         start=True, stop=True)
                        nc.vector.tensor_scalar(
                            out=p2bT_chunks[next_chunk[0]][:, :k],
                            in0=psA[:, :k], scalar1=bcol_sb[:],
                            scalar2=None, op0=ALU.add)
                        next_chunk[0] += 1
                n_win = WPC if max_windows is None else min(max_windows, WPC)

                def emit_ln(w, agg, ot_batch):
                    """Residual + LayerNorm + batched store for window w."""
                    tw = int(T_w[w])
                    nf_w = wt.tile([128, 128], F32, tag="nfw")
                    nc.scalar.dma_start(out=nf_w[:],
                                        in_=nf_shard[w * 128:(w + 1) * 128, :])
                    x = wt.tile([128, 128], F32, tag="x")
                    s1 = wt.tile([128, 1], F32, tag="s1")
                    if tw > 0:
                        nc.vector.scalar_tensor_tensor(
                            out=x[:], in0=agg[:], scalar=0.0, in1=nf_w[:],
                            op0=ALU.add, op1=ALU.add, accum_out=s1[:])
                    else:
                        nc.vector.scalar_tensor_tensor(
                            out=x[:], in0=nf_w[:], scalar=0.0, in1=nf_w[:],
                            op0=ALU.mult, op1=ALU.add, accum_out=s1[:])
                    xsq = wt.tile([128, 128], F32, tag="xsq")
                    s2 = wt.tile([128, 1], F32, tag="s2")
                    nc.vector.scalar_tensor_tensor(
                        out=xsq[:], in0=x[:], scalar=1.0, in1=x[:],
                        op0=ALU.mult, op1=ALU.mult, accum_out=s2[:])
                    mu = wt.tile([128, 1], F32, tag="mu")
                    nc.vector.tensor_scalar(
                        out=mu[:], in0=s1[:], scalar1=1.0 / 128.0,
                        scalar2=None, op0=ALU.mult)
                    mu2 = wt.tile([128, 1], F32, tag="mu2")
                    nc.vector.tensor_mul(out=mu2[:], in0=mu[:], in1=mu[:])
                    var = wt.tile([128, 1], F32, tag="var")
                    nc.vector.tensor_scalar(
                        out=var[:], in0=s2[:], scalar1=1.0 / 128.0,
                        scalar2=mu2[:], op0=ALU.mult, op1=ALU.subtract)
                    sd = wt.tile([128, 1], F32, tag="sd")
                    nc.scalar.activation(out=sd[:], in_=var[:], func=AF.Sqrt,
                                         bias=eps_sb[:], scale=1.0)
                    rs = wt.tile([128, 1], F32, tag="rs")
                    nc.vector.reciprocal(out=rs[:], in_=sd[:])
                    if ln_trivial:
                        nc.vector.tensor_scalar(
                            out=ot_batch[:, w % 4, :], in0=x[:],
                            scalar1=mu[:], scalar2=rs[:],
                            op0=ALU.subtract, op1=ALU.mult)
                    else:
                        xn = wt.tile([128, 128], F32, tag="xn")
                        nc.vector.tensor_scalar(
                            out=xn[:], in0=x[:],
                            scalar1=mu[:], scalar2=rs[:],
                            op0=ALU.subtract, op1=ALU.mult)
                        xw = wt.tile([128, 128], F32, tag="xw")
                        nc.vector.tensor_mul(out=xw[:], in0=xn[:], in1=lnw_sb[:])
                        nc.vector.tensor_add(out=ot_batch[:, w % 4, :],
                                             in0=xw[:], in1=lnb_sb[:])
                    if w % 4 == 3 or w == n_win - 1:
                        w0 = (w // 4) * 4
                        kw = w - w0 + 1
                        dst = out_shard[w0 * 128:(w0 + kw) * 128, :].rearrange(
                            "(t p) h -> p t h", p=128)
                        nc.scalar.dma_start(out=dst, in_=ot_batch[:, :kw, :])

                # software pipeline: agg matmuls of group i emitted after the
                # accumulation matmuls of group i+1 so the PE queue never
                # stalls at an agg waiting on ACT's relu
                pending = None       # (w, agg, S, msg, c0, k, t0, tw, ot)

                def flush_pending():
                    nonlocal pending
                    if pending is None:
                        return
                    (pw, agg, S, msg, c0, k, t0, ptw, pot) = pending
                    for t in range(k):
                        nc.tensor.matmul(
                            out=agg[:], lhsT=S[:, c0 + t, :], rhs=msg[:, t, :],
                            start=(t0 + t == 0), stop=(t0 + t == ptw - 1),
                            skip_group_check=True)
                    pending = None
                    if c0 + k == ptw:
                        emit_ln(pw, agg, pot)

                es_sb = None
                ot_batch = None
                es0_subs = None
                for w in range(n_win):
                    tw = int(T_w[w])
                    tb = int(tile_base[w])
                    if w == 0:
                        # window 0 es in two halves so PE starts sooner;
                        # remaining nf_loc pieces + bulk indices after
                        half = (tw + 1) // 2
                        es0_subs = []
                        for si in (0, half):
                            sn = min(half, tw - si)
                            sub = espool.tile([128, 2 * half * 128], BF16,
                                              name=f"es0s{si}")
                            nc.sync.dma_start(
                                out=sub[:, :2 * sn * 128],
                                in_=esT[:, 2 * si * 128:2 * (si + sn) * 128])
                            es0_subs.append((sub, si, sn))
                    elif w in batch_of and batch_of[w][1] == w:
                        # load es for this batch's windows
                        bi, bw = batch_of[w]
                        bt = T2[bi]
                        es_sb = espool.tile([128, 2 * T2_MAX * 128], BF16,
                                            tag="es")
                        nc.sync.dma_start(
                            out=es_sb[:, :2 * bt * 128],
                            in_=esT[:, 2 * tb * 128:2 * (tb + bt) * 128])
                        es_off = 0
                    else:
                        es_off = 2 * int(T_w[w - 1]) * 128
                    if 1 <= w <= 3:
                        # deferred bulk loads, not needed until window 8/16+
                        pi = w
                        pk = min(NFL_P, NPC_PAD - pi * NFL_P)
                        if pk > 0:
                            nc.sync.dma_start(
                                out=nfl_ps[pi][:],
                                in_=nfT_loc[:, pi * NFL_P:pi * NFL_P + pk])
                        if w == 3:
                            nc.sync.dma_start(out=idxr_b[:],
                                              in_=idx_rank[:, IDX_SPLIT:])

                    # output batch tile (4 windows per store)
                    if w % 4 == 0:
                        ot_batch = outp.tile([128, 4, 128], F32, tag="ot")

                    if tw == 0:
                        flush_pending()
                        emit_ln(w, None, ot_batch)
                        continue

                    # phase B chunks needed by this window's gather (+1 ahead)
                    emit_pb_chunks(w // 4 + 1)

                    chunk = p2bT_chunks[(w * 128) // 512]
                    coff = (w * 128) % 512
                    if w < W_SPLIT:
                        idxs_ap = idxr_a[:, tb * 8:(tb + tw) * 8]
                    else:
                        idxs_ap = idxr_b[:, tb * 8 - IDX_SPLIT:
                                         (tb + tw) * 8 - IDX_SPLIT]
                    p2x = p2xp.tile([128, T_MAX * 128], F32, tag="p2x")
                    nc.gpsimd.ap_gather(
                        out_ap=p2x[:, :tw * 128].rearrange(
                            "p (n d) -> p n d", d=1),
                        in_ap=chunk[:, coff:coff + 128].rearrange(
                            "p (n d) -> p n d", d=1),
                        idxs_ap=idxs_ap,
                        channels=128, num_elems=128, d=1,
                        num_idxs=tw * 128)

                    # S depends only on consts: generate whole window early
                    S = msgs.tile([128, T_MAX, 128], BF16, tag="S")
                    for t in range(tw):
                        nc.vector.tensor_scalar(
                            out=S[:, t, :], in0=iota_sb[:],
                            scalar1=(rankT_a[:, tb + t:tb + t + 1]
                                     if w < W_SPLIT else
                                     rankT_b[:, tb + t - IDX_SPLIT // 8:
                                             tb + t + 1 - IDX_SPLIT // 8]),
                            scalar2=None, op0=ALU.is_equal)

                    def es_ap(g, block):
                        """AP for tile g's ef (block=0) / nfs (block=1)."""
                        if w == 0:
                            for sub, si, sn in es0_subs:
                                if si <= g < si + sn:
                                    o = 2 * (g - si) + block
                                    return sub[:, o * 128:(o + 1) * 128]
                            raise AssertionError
                        o = es_off + (2 * g + block) * 128
                        return es_sb[:, o:o + 128]

                    agg = aggp.tile([128, 128], F32, tag="agg")
                    t_done = 0
                    for c0 in range(0, tw, 4):
                        k = min(4, tw - c0)
                        ps = ppsum.tile([128, 4, 128], F32, tag="ps")
                        for t in range(k):
                            g = c0 + t
                            nc.tensor.matmul(
                                out=ps[:, t, :],
                                lhsT=es_ap(g, 1),
                                rhs=W1_sb[:],
                                start=True, stop=False,
                                skip_group_check=True)
                            nc.tensor.matmul(
                                out=ps[:, t, :],
                                lhsT=es_ap(g, 0),
                                rhs=W3_sb[:],
                                start=False, stop=False,
                                skip_group_check=True)
                            nc.tensor.matmul(
                                out=ps[:, t, :],
                                lhsT=p2x[:, g * 128:(g + 1) * 128],
                                rhs=ident_sb[:],
                                is_transpose=True,
                                start=False, stop=True,
                                skip_group_check=True)
                        flush_pending()
                        msg = msgs.tile([128, 4, 128], BF16, tag="msg")
                        nc.scalar.activation(
                            out=msg[:, :k, :], in_=ps[:, :k, :],
                            func=AF.Relu, scale=1.0)
                        pending = (w, agg, S, msg, c0, k, t_done, tw, ot_batch)
                        t_done += k
                flush_pending()

    nc.compile()
    return nc


# ----------------------------------------------------------------------------
# Full entry: host prep + device run + assembly
# ----------------------------------------------------------------------------

def run(node_features, senders, receivers, edge_features, W, b, ln_w, ln_b,
        n_cores=8, return_nc=False):
    from concourse.bass_utils import run_bass_kernel_spmd
    st, in_maps = host_prep(node_features, senders, receivers, edge_features,
                            W, b, ln_w, ln_b, n_cores)
    nc = build_kernel(st)
    res = run_bass_kernel_spmd(nc, in_maps, core_ids=list(range(n_cores)))
    N = st["N"]
    out = np.empty((N, 128), np.float32)
    for c in range(n_cores):
        shard = res.results[c]["out_shard"]
        for j, wg in enumerate(st["core_wins"][c]):
            lo, hi = wg * 128, min((wg + 1) * 128, N)
            out[lo:hi] = shard[j * 128:j * 128 + hi - lo]
    if return_nc:
        return out, nc, st, in_maps
    return out


# ----------------------------------------------------------------------------
# Harness entry point
# ----------------------------------------------------------------------------

def kernel(**inputs):
    """Full-input entry: shards across 8 NeuronCores internally."""
    out = run(
        node_features=np.asarray(inputs["node_features"], np.float32),
        senders=np.asarray(inputs["senders"], np.int32),
        receivers=np.asarray(inputs["receivers"], np.int32),
        edge_features=np.asarray(inputs["edge_features"], np.float32),
        W=np.asarray(inputs["W"], np.float32),
        b=np.asarray(inputs["b"], np.float32),
        ln_w=np.asarray(inputs["ln_w"], np.float32),
        ln_b=np.asarray(inputs["ln_b"], np.float32),
        n_cores=8,
    )
    return out.astype(np.float32)
